# revision 25
# baseline (speedup 1.0000x reference)
"""MoE kernel v7: F/8 expert-slice scheme with paced weight streaming.

Each core holds a 512-wide F-slice of ALL 8 experts and processes ALL
16384 token-assignment columns (zero load-balance padding, perfectly
SPMD); its mm2 output is a partial over its F-slice and the 8 partials
are summed on host.

Since each expert's tiles are consecutive in the column stream, only 2
experts' weight slices need to be SBUF-resident at a time. Weights live
in a 3-deep pool of per-expert tiles; expert g+2's DMA is issued at the
first tile of group g, so the pool's WAR hazard paces the weight
traffic across the whole kernel instead of frontloading 17 MB (which
starved HBM bandwidth shared with the chip-neighbor core). x tiles
stream on the Activation engine's DMA queues (6-deep prefetch) while
weights and y use the Sync engine's queues.

Per tile (tw cols): mm1 = 4 fo x 8 ko matmuls -> gelu(+b1) -> h; mm2 =
8 do x 4 fi matmuls -> cast bf16 -> y tile. Issue order pipelines
mm1(t+1) before mm2(t) so gelu latency never stalls the PE. w1 DRAM is
laid out per-(expert, fq) so a 256 KB chunk unblocks the first matmul
chain; x0 is split into two half-DMAs for the same reason. DMA issue
instructions cost ~700 ns each, so transfers are kept big and few.

DRAM layouts per core (FL = F/8 = 512, FQ = FL/128 = 4):
  x   [NT, 128, KO, 512]    bf16  x[t][p, ko, c] = xf[ids[c], ko*128+p]
  w1  [E, 128, FQ, KO, 128] bf16  w1[e][p, fq, ko, j] = w1_e[ko*128+p, cF*FL+fq*128+j]
  w2  [E, 128, FQ, D]       bf16  w2[e][p, fi, d]     = w2_e[cF*FL+fi*128+p, d]
  b1  [128, E*FQ]           f32   b1[p, e*4+fq]       = b1_e[cF*FL+fq*128+p]
  y   [NT, 128, KO, 512]    bf16  partial (gelu(x@w1l+b1l) @ w2l)^T
(cF = the core id = which F-slice it owns.)
"""

import numpy as np
import ml_dtypes

N_CORES = 8
D = 1024
F = 4096
E = 8
KO = D // 128
FL = F // N_CORES     # 512 local F columns per core
FQ = FL // 128        # 4 local f-chunks
CT = 512

BF16 = ml_dtypes.bfloat16

_NC_CACHE: dict[tuple, object] = {}
LAST_RESULTS = None


def _expert_tiles(c):
    """Split c columns into tile widths <=512, all >=256 when possible."""
    if c == 0:
        return []
    n_full, rem = divmod(c, CT)
    if rem == 0:
        return [CT] * n_full
    if rem >= 256 or n_full == 0:
        return [CT] * n_full + [rem]
    a = (CT + rem) // 2
    a += a & 1
    return [CT] * (n_full - 1) + [a, CT + rem - a]


def _build(spec):
    import concourse.mybir as mybir
    from concourse import bacc
    from concourse.tile import TileContext

    fp32 = mybir.dt.float32
    bf16 = mybir.dt.bfloat16
    Gelu = mybir.ActivationFunctionType.Gelu

    NT = len(spec)
    # Expert groups: tiles of one expert are consecutive.
    e_list = []
    first_tile = []           # first tile index of each group
    for ti, (e, _) in enumerate(spec):
        if not e_list or e != e_list[-1]:
            e_list.append(e)
            first_tile.append(ti)
    NG = len(e_list)
    gidx = []                 # group index per tile
    g = -1
    for ti in range(NT):
        if g + 1 < NG and first_tile[g + 1] == ti:
            g += 1
        gidx.append(g)

    nc = bacc.Bacc(
        "TRN2", target_bir_lowering=False, debug=False, num_devices=N_CORES
    )
    x = nc.dram_tensor("x", [NT, 128, KO, CT], bf16, kind="ExternalInput")
    w1 = nc.dram_tensor("w1", [E, 128, FQ, KO, 128], bf16, kind="ExternalInput")
    w2 = nc.dram_tensor("w2", [E, 128, FQ, D], bf16, kind="ExternalInput")
    b1 = nc.dram_tensor("b1", [128, E * FQ], fp32, kind="ExternalInput")
    y = nc.dram_tensor("y", [NT, 128, KO, CT], bf16, kind="ExternalOutput")

    with TileContext(nc) as tc:
        with (
            tc.tile_pool(name="bpool", bufs=1) as bpool,
            tc.tile_pool(name="wepool", bufs=3) as wepool,
            tc.tile_pool(name="xpool", bufs=6) as xpool,
            tc.tile_pool(name="hpool", bufs=3) as hpool,
            tc.tile_pool(name="ypool", bufs=3) as ypool,
            tc.tile_pool(name="ph", bufs=4, space="PSUM") as phpool,
            tc.tile_pool(name="py", bufs=4, space="PSUM") as pypool,
        ):
            b1_sb = bpool.tile([128, E * FQ], fp32)

            w1t: dict[int, object] = {}
            w2t: dict[int, object] = {}

            def issue_wdma(gi):
                if gi >= NG:
                    return
                e = e_list[gi]
                t1 = wepool.tile([128, FQ, KO, 128], bf16,
                                 tag="w1e", name="w1t")
                nc.sync.dma_start(t1[:, 0:2], w1[e][:, 0:2])
                nc.sync.dma_start(t1[:, 2:4], w1[e][:, 2:4])
                t2 = wepool.tile([128, FQ, D], bf16, tag="w2e", name="w2t")
                nc.sync.dma_start(t2[:], w2[e])
                w1t[gi], w2t[gi] = t1, t2

            # Head DMAs: the first mm1 chain accumulates over ko, so x0
            # is transferred as 8 per-ko 128 KB chunks — the chain's MM
            # for chunk ko starts as soon as that chunk lands (~1 us per
            # chunk), turning the head DMA wait into real PE work that
            # also warms the HAM clock gate. The first expert's weights
            # stream concurrently on the Activation engine's queues,
            # finest-grained chunks (fq0 halves) first. Early DMA
            # bandwidth is fair-shared across in-flight queues, so
            # everything else staggers behind in need order.
            NX0 = min(6, NT)
            x_tiles = [xpool.tile([128, KO, CT], bf16, tag="x_sb",
                                  name="x_sb")
                       for _ in range(NX0)]
            e0 = e_list[0]
            w1t[0] = wepool.tile([128, FQ, KO, 128], bf16,
                                 tag="w1e", name="w1t")
            w2t[0] = wepool.tile([128, FQ, D], bf16, tag="w2e", name="w2t")
            for ko in range(KO):
                nc.sync.dma_start(x_tiles[0][:, ko], x[0][:, ko])
            if NT > 1:
                nc.sync.dma_start(x_tiles[1][:, 0:4], x[1][:, 0:4])
                nc.sync.dma_start(x_tiles[1][:, 4:8], x[1][:, 4:8])
            for i in range(2, min(4, NT)):
                nc.sync.dma_start(x_tiles[i][:], x[i])
            issue_wdma(1)
            nc.scalar.dma_start(w1t[0][:, 0, 0:4], w1[e0][:, 0, 0:4])
            nc.scalar.dma_start(w1t[0][:, 0, 4:8], w1[e0][:, 0, 4:8])
            nc.scalar.dma_start(b1_sb[:], b1[:])
            nc.scalar.dma_start(w1t[0][:, 1], w1[e0][:, 1])
            nc.scalar.dma_start(w1t[0][:, 2:4], w1[e0][:, 2:4])
            nc.scalar.dma_start(w2t[0][:], w2[e0])
            for i in range(4, NX0):
                nc.scalar.dma_start(x_tiles[i][:], x[i])

            def mm1(t):
                e, tw = spec[t]
                gi = gidx[t]
                x_sb = x_tiles[t]
                h_sb = hpool.tile([128, FQ, CT], bf16, tag="h_sb")
                for fo in range(FQ):
                    ph = phpool.tile([128, CT], fp32, tag="ph")
                    for ko in range(KO):
                        nc.tensor.matmul(
                            ph[:, :tw],
                            lhsT=w1t[gi][:, fo, ko],
                            rhs=x_sb[:, ko, :tw],
                            start=(ko == 0),
                            stop=(ko == KO - 1),
                        )
                    nc.scalar.activation(
                        h_sb[:, fo, :tw],
                        ph[:, :tw],
                        Gelu,
                        bias=b1_sb[:, e * FQ + fo: e * FQ + fo + 1],
                    )
                return h_sb

            def mm2(t, h_sb):
                e, tw = spec[t]
                gi = gidx[t]
                y_sb = ypool.tile([128, KO, CT], bf16, tag="y_sb")
                for do in range(KO):
                    py = pypool.tile([128, CT], fp32, tag="py")
                    for fi in range(FQ):
                        nc.tensor.matmul(
                            py[:, :tw],
                            lhsT=w2t[gi][:, fi, do * 128:(do + 1) * 128],
                            rhs=h_sb[:, fi, :tw],
                            start=(fi == 0),
                            stop=(fi == FQ - 1),
                        )
                    nc.vector.tensor_copy(y_sb[:, do, :tw], py[:, :tw])
                    if t == NT - 1:
                        # Tail: per-do writeback so the kernel doesn't end
                        # on a full 1 MB transfer.
                        nc.sync.dma_start(y[t][:, do, :], y_sb[:, do, :])
                if t != NT - 1:
                    nc.sync.dma_start(y[t], y_sb[:])

            h_prev = mm1(0)
            for t in range(NT):
                if t + 1 < NT:
                    h_next = mm1(t + 1)
                else:
                    h_next = None
                if t + 6 < NT:
                    x_sb = xpool.tile([128, KO, CT], bf16, tag="x_sb",
                                      name="x_sb")
                    nc.scalar.dma_start(x_sb[:], x[t + 6])
                    x_tiles.append(x_sb)
                g = gidx[t]
                if first_tile[g] == t:
                    # Start streaming the weights needed 2 expert-groups
                    # from now; the wepool WAR hazard paces it.
                    issue_wdma(g + 2)
                mm2(t, h_prev)
                h_prev = h_next

    nc.compile()
    return nc


def kernel(x, gate_w, w1, b1, w2, b2):
    from concourse.bass_utils import run_bass_kernel_spmd

    global LAST_RESULTS

    x = np.asarray(x, dtype=np.float32)
    gate_w = np.asarray(gate_w, dtype=np.float32)
    w1 = np.asarray(w1, dtype=np.float32)
    b1 = np.asarray(b1, dtype=np.float32)
    w2 = np.asarray(w2, dtype=np.float32)
    b2 = np.asarray(b2, dtype=np.float32)

    B, S, Din = x.shape
    assert Din == D and gate_w.shape == (D, E)
    T = B * S
    xf = x.reshape(T, D)

    # ---- Host router + dispatch ----
    logits = xf.astype(np.float64) @ gate_w.astype(np.float64)
    idx0 = np.argmax(logits, axis=1)
    rows = np.arange(T)
    v0 = logits[rows, idx0]
    l2 = logits.copy()
    l2[rows, idx0] = -np.inf
    idx1 = np.argmax(l2, axis=1)
    v1_ = l2[rows, idx1]
    e1 = np.exp(v1_ - v0)
    cw0 = 1.0 / (1.0 + e1)
    cw1 = e1 / (1.0 + e1)

    token_ids = []
    combine_w = []
    for e in range(E):
        sel0 = idx0 == e
        sel1 = idx1 == e
        ids = np.nonzero(sel0 | sel1)[0]
        w = np.where(sel0[ids], cw0[ids], cw1[ids])
        token_ids.append(ids)
        combine_w.append(w)

    # ---- Tile spec: per-expert tiles over the global column stream ----
    spec = []            # (expert, tile_width)
    tile_seg = []        # (expert, start offset into token_ids[e]) per tile
    for e in range(E):
        off = 0
        for tw in _expert_tiles(len(token_ids[e])):
            spec.append((e, tw))
            tile_seg.append((e, off))
            off += tw
    NT = len(spec)

    key = tuple(spec)
    if key not in _NC_CACHE:
        _NC_CACHE[key] = _build(spec)
    nc = _NC_CACHE[key]

    # ---- Shared x dispatch (same array for every core) ----
    xtiles = np.zeros((NT, 128, KO, CT), dtype=BF16)
    for ti, ((e, tw), (_, off)) in enumerate(zip(spec, tile_seg)):
        ids_seg = token_ids[e][off: off + tw]
        blk = xf[ids_seg].astype(BF16).reshape(tw, KO, 128).transpose(2, 1, 0)
        xtiles[ti, :, :, :tw] = blk
    xtiles = np.ascontiguousarray(xtiles)

    # ---- Per-core weight F-slices ----
    in_maps = []
    for c in range(N_CORES):
        sl = slice(c * FL, (c + 1) * FL)
        w1c = np.stack(
            [w1[e][:, sl].reshape(KO, 128, FQ, 128).transpose(1, 2, 0, 3)
             for e in range(E)]
        ).astype(BF16)                       # [E, 128, FQ, KO, 128]
        w2c = np.stack(
            [w2[e][sl, :].reshape(FQ, 128, D).transpose(1, 0, 2)
             for e in range(E)]
        ).astype(BF16)                       # [E, 128, FQ, D]
        b1c = np.concatenate(
            [b1[e][sl].reshape(FQ, 128).T for e in range(E)], axis=1
        )                                    # [128, E*FQ]
        in_maps.append({
            "x": xtiles,
            "w1": np.ascontiguousarray(w1c),
            "w2": np.ascontiguousarray(w2c),
            "b1": np.ascontiguousarray(b1c),
        })

    res = run_bass_kernel_spmd(nc, in_maps, core_ids=list(range(N_CORES)))
    LAST_RESULTS = res

    # ---- Host: sum F-slice partials, combine, scatter ----
    ysum = np.zeros((NT, 128, KO, CT), dtype=np.float32)
    for c in range(N_CORES):
        ysum += res.results[c]["y"].astype(np.float32)

    out = np.zeros((T, D), dtype=np.float32)
    for ti, ((e, tw), (_, off)) in enumerate(zip(spec, tile_seg)):
        ids_seg = token_ids[e][off: off + tw]
        cw_seg = combine_w[e][off: off + tw].astype(np.float32)
        yt = ysum[ti, :, :, :tw].transpose(2, 1, 0).reshape(tw, D)
        out[ids_seg] += cw_seg[:, None] * (yt + b2[e])

    return out.reshape(B, S, D)


# revision 26
# speedup vs baseline: 1.0423x; 1.0423x over previous
"""MoE kernel v7: F/8 expert-slice scheme with paced weight streaming.

Each core holds a 512-wide F-slice of ALL 8 experts and processes ALL
16384 token-assignment columns (zero load-balance padding, perfectly
SPMD); its mm2 output is a partial over its F-slice and the 8 partials
are summed on host.

Since each expert's tiles are consecutive in the column stream, only 2
experts' weight slices need to be SBUF-resident at a time. Weights live
in a 3-deep pool of per-expert tiles; expert g+2's DMA is issued at the
first tile of group g, so the pool's WAR hazard paces the weight
traffic across the whole kernel instead of frontloading 17 MB (which
starved HBM bandwidth shared with the chip-neighbor core). x tiles
stream on the Activation engine's DMA queues (6-deep prefetch) while
weights and y use the Sync engine's queues.

Per tile (tw cols): mm1 = 4 fo x 8 ko matmuls -> gelu(+b1) -> h; mm2 =
8 do x 4 fi matmuls -> cast bf16 -> y tile. Issue order pipelines
mm1(t+1) before mm2(t) so gelu latency never stalls the PE. w1 DRAM is
laid out per-(expert, fq) so a 256 KB chunk unblocks the first matmul
chain; x0 is split into two half-DMAs for the same reason. DMA issue
instructions cost ~700 ns each, so transfers are kept big and few.

DRAM layouts per core (FL = F/8 = 512, FQ = FL/128 = 4):
  x   [NT, 128, KO, 512]    bf16  x[t][p, ko, c] = xf[ids[c], ko*128+p]
  w1  [E, 128, FQ, KO, 128] bf16  w1[e][p, fq, ko, j] = w1_e[ko*128+p, cF*FL+fq*128+j]
  w2  [E, 128, FQ, D]       bf16  w2[e][p, fi, d]     = w2_e[cF*FL+fi*128+p, d]
  b1  [128, E*FQ]           f32   b1[p, e*4+fq]       = b1_e[cF*FL+fq*128+p]
  y   [NT, 128, KO, 512]    bf16  partial (gelu(x@w1l+b1l) @ w2l)^T
(cF = the core id = which F-slice it owns.)
"""

import numpy as np
import ml_dtypes

N_CORES = 8
D = 1024
F = 4096
E = 8
KO = D // 128
FL = F // N_CORES     # 512 local F columns per core
FQ = FL // 128        # 4 local f-chunks
CT = 512

BF16 = ml_dtypes.bfloat16

_NC_CACHE: dict[tuple, object] = {}
LAST_RESULTS = None


def _expert_tiles(c):
    """Split c columns into tile widths <=512, all >=256 when possible."""
    if c == 0:
        return []
    n_full, rem = divmod(c, CT)
    if rem == 0:
        return [CT] * n_full
    if rem >= 256 or n_full == 0:
        return [CT] * n_full + [rem]
    a = (CT + rem) // 2
    a += a & 1
    return [CT] * (n_full - 1) + [a, CT + rem - a]


def _build(spec):
    import concourse.mybir as mybir
    from concourse import bacc
    from concourse.tile import TileContext

    fp32 = mybir.dt.float32
    bf16 = mybir.dt.bfloat16
    Gelu = mybir.ActivationFunctionType.Gelu

    NT = len(spec)
    # Expert groups: tiles of one expert are consecutive.
    e_list = []
    first_tile = []           # first tile index of each group
    for ti, (e, _) in enumerate(spec):
        if not e_list or e != e_list[-1]:
            e_list.append(e)
            first_tile.append(ti)
    NG = len(e_list)
    gidx = []                 # group index per tile
    g = -1
    for ti in range(NT):
        if g + 1 < NG and first_tile[g + 1] == ti:
            g += 1
        gidx.append(g)

    nc = bacc.Bacc(
        "TRN2", target_bir_lowering=False, debug=False, num_devices=N_CORES
    )
    x = nc.dram_tensor("x", [NT, 128, KO, CT], bf16, kind="ExternalInput")
    w1 = nc.dram_tensor("w1", [E, 128, FQ, KO, 128], bf16, kind="ExternalInput")
    w2 = nc.dram_tensor("w2", [E, 128, FQ, D], bf16, kind="ExternalInput")
    b1 = nc.dram_tensor("b1", [128, E * FQ], fp32, kind="ExternalInput")
    y = nc.dram_tensor("y", [NT, 128, KO, CT], bf16, kind="ExternalOutput")

    with TileContext(nc) as tc:
        with (
            tc.tile_pool(name="bpool", bufs=1) as bpool,
            tc.tile_pool(name="wepool", bufs=3) as wepool,
            tc.tile_pool(name="xpool", bufs=6) as xpool,
            tc.tile_pool(name="hpool", bufs=3) as hpool,
            tc.tile_pool(name="ypool", bufs=3) as ypool,
            tc.tile_pool(name="ph", bufs=4, space="PSUM") as phpool,
            tc.tile_pool(name="py", bufs=4, space="PSUM") as pypool,
        ):
            b1_sb = bpool.tile([128, E * FQ], fp32)

            w1t: dict[int, object] = {}
            w2t: dict[int, object] = {}

            def issue_wdma(gi):
                if gi >= NG:
                    return
                e = e_list[gi]
                t1 = wepool.tile([128, FQ, KO, 128], bf16,
                                 tag="w1e", name="w1t")
                nc.sync.dma_start(t1[:, 0:2], w1[e][:, 0:2])
                nc.sync.dma_start(t1[:, 2:4], w1[e][:, 2:4])
                t2 = wepool.tile([128, FQ, D], bf16, tag="w2e", name="w2t")
                nc.sync.dma_start(t2[:], w2[e])
                w1t[gi], w2t[gi] = t1, t2

            # Head DMAs: early DMA bandwidth is fair-shared across all
            # in-flight queues, so the critical first transfers (x0 halves
            # + the first fq of the first expert's w1, 768 KB total) go
            # alone first; everything else is staggered behind in need
            # order. Later x tiles stream on the Activation engine's
            # queues so the paced weight DMAs on Sync can never block
            # them.
            NX0 = min(6, NT)
            x_tiles = [xpool.tile([128, KO, CT], bf16, tag="x_sb",
                                  name="x_sb")
                       for _ in range(NX0)]
            e0 = e_list[0]
            w1t[0] = wepool.tile([128, FQ, KO, 128], bf16,
                                 tag="w1e", name="w1t")
            w2t[0] = wepool.tile([128, FQ, D], bf16, tag="w2e", name="w2t")
            nc.sync.dma_start(x_tiles[0][:, 0:4], x[0][:, 0:4])
            nc.sync.dma_start(x_tiles[0][:, 4:8], x[0][:, 4:8])
            nc.sync.dma_start(w1t[0][:, 0], w1[e0][:, 0])
            if NT > 1:
                nc.sync.dma_start(x_tiles[1][:, 0:4], x[1][:, 0:4])
                nc.sync.dma_start(x_tiles[1][:, 4:8], x[1][:, 4:8])
            nc.sync.dma_start(w1t[0][:, 1], w1[e0][:, 1])
            nc.sync.dma_start(w1t[0][:, 2:4], w1[e0][:, 2:4])
            nc.sync.dma_start(w2t[0][:], w2[e0])
            nc.sync.dma_start(b1_sb[:], b1[:])
            for i in range(2, min(4, NT)):
                nc.sync.dma_start(x_tiles[i][:], x[i])
            issue_wdma(1)
            for i in range(4, NX0):
                nc.scalar.dma_start(x_tiles[i][:], x[i])

            def mm1(t):
                e, tw = spec[t]
                gi = gidx[t]
                x_sb = x_tiles[t]
                h_sb = hpool.tile([128, FQ, CT], bf16, tag="h_sb")
                for fo in range(FQ):
                    ph = phpool.tile([128, CT], fp32, tag="ph")
                    for ko in range(KO):
                        nc.tensor.matmul(
                            ph[:, :tw],
                            lhsT=w1t[gi][:, fo, ko],
                            rhs=x_sb[:, ko, :tw],
                            start=(ko == 0),
                            stop=(ko == KO - 1),
                        )
                    nc.scalar.activation(
                        h_sb[:, fo, :tw],
                        ph[:, :tw],
                        Gelu,
                        bias=b1_sb[:, e * FQ + fo: e * FQ + fo + 1],
                    )
                return h_sb

            def mm2(t, h_sb):
                e, tw = spec[t]
                gi = gidx[t]
                y_sb = ypool.tile([128, KO, CT], bf16, tag="y_sb")
                for do in range(KO):
                    py = pypool.tile([128, CT], fp32, tag="py")
                    for fi in range(FQ):
                        nc.tensor.matmul(
                            py[:, :tw],
                            lhsT=w2t[gi][:, fi, do * 128:(do + 1) * 128],
                            rhs=h_sb[:, fi, :tw],
                            start=(fi == 0),
                            stop=(fi == FQ - 1),
                        )
                    nc.vector.tensor_copy(y_sb[:, do, :tw], py[:, :tw])
                    if t == NT - 1:
                        # Tail: per-do writeback so the kernel doesn't end
                        # on a full 1 MB transfer.
                        nc.sync.dma_start(y[t][:, do, :], y_sb[:, do, :])
                if t != NT - 1:
                    nc.sync.dma_start(y[t], y_sb[:])

            h_prev = mm1(0)
            for t in range(NT):
                if t + 1 < NT:
                    h_next = mm1(t + 1)
                else:
                    h_next = None
                if t + 6 < NT:
                    x_sb = xpool.tile([128, KO, CT], bf16, tag="x_sb",
                                      name="x_sb")
                    nc.scalar.dma_start(x_sb[:], x[t + 6])
                    x_tiles.append(x_sb)
                g = gidx[t]
                if first_tile[g] == t:
                    # Start streaming the weights needed 2 expert-groups
                    # from now; the wepool WAR hazard paces it.
                    issue_wdma(g + 2)
                mm2(t, h_prev)
                h_prev = h_next

    nc.compile()
    return nc


def kernel(x, gate_w, w1, b1, w2, b2):
    from concourse.bass_utils import run_bass_kernel_spmd

    global LAST_RESULTS

    x = np.asarray(x, dtype=np.float32)
    gate_w = np.asarray(gate_w, dtype=np.float32)
    w1 = np.asarray(w1, dtype=np.float32)
    b1 = np.asarray(b1, dtype=np.float32)
    w2 = np.asarray(w2, dtype=np.float32)
    b2 = np.asarray(b2, dtype=np.float32)

    B, S, Din = x.shape
    assert Din == D and gate_w.shape == (D, E)
    T = B * S
    xf = x.reshape(T, D)

    # ---- Host router + dispatch ----
    logits = xf.astype(np.float64) @ gate_w.astype(np.float64)
    idx0 = np.argmax(logits, axis=1)
    rows = np.arange(T)
    v0 = logits[rows, idx0]
    l2 = logits.copy()
    l2[rows, idx0] = -np.inf
    idx1 = np.argmax(l2, axis=1)
    v1_ = l2[rows, idx1]
    e1 = np.exp(v1_ - v0)
    cw0 = 1.0 / (1.0 + e1)
    cw1 = e1 / (1.0 + e1)

    token_ids = []
    combine_w = []
    for e in range(E):
        sel0 = idx0 == e
        sel1 = idx1 == e
        ids = np.nonzero(sel0 | sel1)[0]
        w = np.where(sel0[ids], cw0[ids], cw1[ids])
        token_ids.append(ids)
        combine_w.append(w)

    # ---- Tile spec: per-expert tiles over the global column stream ----
    spec = []            # (expert, tile_width)
    tile_seg = []        # (expert, start offset into token_ids[e]) per tile
    for e in range(E):
        off = 0
        for tw in _expert_tiles(len(token_ids[e])):
            spec.append((e, tw))
            tile_seg.append((e, off))
            off += tw
    NT = len(spec)

    key = tuple(spec)
    if key not in _NC_CACHE:
        _NC_CACHE[key] = _build(spec)
    nc = _NC_CACHE[key]

    # ---- Shared x dispatch (same array for every core) ----
    xtiles = np.zeros((NT, 128, KO, CT), dtype=BF16)
    for ti, ((e, tw), (_, off)) in enumerate(zip(spec, tile_seg)):
        ids_seg = token_ids[e][off: off + tw]
        blk = xf[ids_seg].astype(BF16).reshape(tw, KO, 128).transpose(2, 1, 0)
        xtiles[ti, :, :, :tw] = blk
    xtiles = np.ascontiguousarray(xtiles)

    # ---- Per-core weight F-slices ----
    in_maps = []
    for c in range(N_CORES):
        sl = slice(c * FL, (c + 1) * FL)
        w1c = np.stack(
            [w1[e][:, sl].reshape(KO, 128, FQ, 128).transpose(1, 2, 0, 3)
             for e in range(E)]
        ).astype(BF16)                       # [E, 128, FQ, KO, 128]
        w2c = np.stack(
            [w2[e][sl, :].reshape(FQ, 128, D).transpose(1, 0, 2)
             for e in range(E)]
        ).astype(BF16)                       # [E, 128, FQ, D]
        b1c = np.concatenate(
            [b1[e][sl].reshape(FQ, 128).T for e in range(E)], axis=1
        )                                    # [128, E*FQ]
        in_maps.append({
            "x": xtiles,
            "w1": np.ascontiguousarray(w1c),
            "w2": np.ascontiguousarray(w2c),
            "b1": np.ascontiguousarray(b1c),
        })

    res = run_bass_kernel_spmd(nc, in_maps, core_ids=list(range(N_CORES)))
    LAST_RESULTS = res

    # ---- Host: sum F-slice partials, combine, scatter ----
    ysum = np.zeros((NT, 128, KO, CT), dtype=np.float32)
    for c in range(N_CORES):
        ysum += res.results[c]["y"].astype(np.float32)

    out = np.zeros((T, D), dtype=np.float32)
    for ti, ((e, tw), (_, off)) in enumerate(zip(spec, tile_seg)):
        ids_seg = token_ids[e][off: off + tw]
        cw_seg = combine_w[e][off: off + tw].astype(np.float32)
        yt = ysum[ti, :, :, :tw].transpose(2, 1, 0).reshape(tw, D)
        out[ids_seg] += cw_seg[:, None] * (yt + b2[e])

    return out.reshape(B, S, D)
